# revision 1
# baseline (speedup 1.0000x reference)
"""Trainium2 Bass kernel for nn_HFMelSpectrogram.

Pipeline (per core, 4 batches of the 32-batch waveform):
  1. STFT-as-GEMM: spec[i, t] = sum_n Wp[i, n] * x[480*t + n], n in [0,1024).
     Host packs the 1024 nontrivial DFT rows (513 cos + 511 sin; the sin rows
     for k=0 and k=512 are identically zero) so the contraction is exactly
     8 x 128.  The frame matrix is supplied as two phase-shifted block
     matrices XtA[j,t] = x[480t+j], XtB[j,t] = x[480t+512+j] (j < 512) so the
     rhs operand needs no on-chip transpose or column shifting.
  2. Square on ScalarE (PSUM -> SBUF, bf16).
  3. Mel projection: melT[t, m] = sum_i sq[i, t] * Mexp[i, m] where Mexp maps
     each packed row back to its mel filter column (power = cos^2 + sin^2 is
     absorbed into the GEMM).  bf16, free dim = 64.
  4. Ln on ScalarE -> logmelT[t, m] (the 10/log(10) scale is folded into R).
  5. Bilinear height-resize 1000 -> 1024 as a banded GEMM: each 128-row
     h-tile draws from at most two 128-row t-tiles.  The 4 batches are packed
     side by side in the free dim (4*64 = 256) so fp32r runs at full rate.
All large matmuls use float32r (full-rate fp32 path on TRN2).
"""

import numpy as np
import ml_dtypes

import concourse.bass as bass
import concourse.bacc as bacc
import concourse.tile as tile
import concourse.mybir as mybir
from concourse.bass_utils import run_bass_kernel_spmd

F32 = mybir.dt.float32
F32R = mybir.dt.float32r
BF16 = mybir.dt.bfloat16

N_FFT = 1024
HOP = 480
NB_MAX = 1000      # frames kept by the reference
N_MELS = 64
SPECW = 1024       # output height after resize
NBINS = 513
B, L = 32, 480000
NCORES = 8
BPC = B // NCORES  # batches per core
TFR = 1024         # padded frame count (frames >= 1000 are zeroed via R)
PAD = N_FFT // 2

# Set by test harness to collect a profile; harness default leaves it off.
TRACE = False
LAST_RESULTS = None


def _resize_blocks():
    """Banded resize matrix blocks, f32 coords matching the reference."""
    scale = np.float32((NB_MAX - 1) / (SPECW - 1))
    pos = np.arange(SPECW, dtype=np.float32) * scale
    h0 = np.clip(np.floor(pos).astype(np.int64), 0, NB_MAX - 1)
    frac = (pos - h0.astype(np.float32)).astype(np.float64)
    h1 = np.minimum(h0 + 1, NB_MAX - 1)
    c = 10.0 / np.log(10.0)
    r = np.zeros((TFR, SPECW), np.float64)
    cols = np.arange(SPECW)
    r[h0, cols] += (1.0 - frac) * c
    r[h1, cols] += frac * c
    blocks = np.zeros((8, 2, 128, 128), np.float32)
    qpairs = []
    for g in range(8):
        sub = r[:, g * 128:(g + 1) * 128]
        rows = np.nonzero(sub.any(axis=1))[0]
        qs = sorted({int(q) for q in rows // 128})
        assert 1 <= len(qs) <= 2, qs
        q0 = qs[0]
        q1 = qs[1] if len(qs) > 1 else min(q0 + 1, 7)
        blocks[g, 0] = sub[q0 * 128:(q0 + 1) * 128].astype(np.float32)
        blocks[g, 1] = sub[q1 * 128:(q1 + 1) * 128].astype(np.float32)
        qpairs.append((q0, q1))
    return blocks, qpairs


_RBLOCKS, _QPAIRS = _resize_blocks()


def _build_bass():
    nc = bacc.Bacc("TRN2", target_bir_lowering=False, debug=False,
                   num_devices=NCORES)
    xa = nc.declare_dram_parameter("xa", [BPC, 4, 128, TFR], F32R, isOutput=False)
    xb = nc.declare_dram_parameter("xb", [BPC, 4, 128, TFR], F32R, isOutput=False)
    wt = nc.declare_dram_parameter("wt", [8, 128, 1024], F32R, isOutput=False)
    mexp = nc.declare_dram_parameter("mexp", [8, 128, N_MELS], BF16, isOutput=False)
    rblk = nc.declare_dram_parameter("rblk", [8, 2, 128, 128], F32R, isOutput=False)
    out = nc.declare_dram_parameter("out", [BPC, SPECW, N_MELS], F32, isOutput=True)

    with tile.TileContext(nc) as tc:
        with (
            tc.tile_pool(name="consts", bufs=1) as consts,
            tc.tile_pool(name="xt", bufs=3) as xpool,
            tc.tile_pool(name="sq", bufs=3) as sqpool,
            tc.tile_pool(name="lm", bufs=1) as lmpool,
            tc.tile_pool(name="ot", bufs=3) as otpool,
            tc.tile_pool(name="specp", bufs=5, space="PSUM") as specp,
            tc.tile_pool(name="melp", bufs=2, space="PSUM") as melp,
            tc.tile_pool(name="resp", bufs=1, space="PSUM") as resp,
        ):
            wt_t = []
            for c in range(8):
                t = consts.tile([128, 1024], F32R, tag=f"wt{c}", name=f"wt{c}")
                nc.gpsimd.dma_start(out=t, in_=wt[c])
                wt_t.append(t)
            mexp_t = []
            for c in range(8):
                t = consts.tile([128, N_MELS], BF16, tag=f"me{c}", name=f"me{c}")
                nc.gpsimd.dma_start(out=t, in_=mexp[c])
                mexp_t.append(t)
            rb_t = []
            for g in range(8):
                pair = []
                for j in range(2):
                    t = consts.tile([128, 128], F32R, tag=f"rb{g}_{j}",
                                    name=f"rb{g}_{j}")
                    nc.gpsimd.dma_start(out=t, in_=rblk[g, j])
                    pair.append(t)
                rb_t.append(pair)
            eps_t = consts.tile([128, 1], F32, tag="eps", name="eps")
            nc.vector.memset(eps_t, 1e-10)

            # logmelT for all 4 batches: [t_local, t_chunk g, 4*64]
            logmel = lmpool.tile([128, 8, BPC * N_MELS], F32R, tag="lm",
                                 name="logmel")

            def issue_mel(sq_tiles, b, tt):
                for s in range(4):
                    g = tt * 4 + s
                    mt = melp.tile([128, N_MELS], F32, tag="mel", name="melpsum")
                    for i in range(8):
                        nc.tensor.matmul(
                            mt,
                            lhsT=sq_tiles[i][:, s * 128:(s + 1) * 128],
                            rhs=mexp_t[i],
                            start=(i == 0),
                            stop=(i == 7),
                        )
                    nc.scalar.activation(
                        out=logmel[:, g, b * N_MELS:(b + 1) * N_MELS],
                        in_=mt,
                        func=mybir.ActivationFunctionType.Ln,
                        bias=eps_t,
                        scale=1.0,
                    )

            pending = None
            for b in range(BPC):
                xt = []
                for c in range(8):
                    t = xpool.tile([128, TFR], F32R, tag=f"xt{c}", name=f"xt{c}")
                    src = xa[b, c] if c < 4 else xb[b, c - 4]
                    nc.sync.dma_start(out=t, in_=src)
                    xt.append(t)
                for tt in range(2):
                    sq_tiles = []
                    for m in range(8):
                        ps = specp.tile([128, 512], F32, tag="spec",
                                        name="specpsum")
                        for c in range(8):
                            nc.tensor.matmul(
                                ps,
                                lhsT=wt_t[c][:, m * 128:(m + 1) * 128],
                                rhs=xt[c][:, tt * 512:(tt + 1) * 512],
                                start=(c == 0),
                                stop=(c == 7),
                            )
                        sq = sqpool.tile([128, 512], BF16, tag=f"sq{m}",
                                         name=f"sq{m}")
                        nc.scalar.square(sq, ps)
                        sq_tiles.append(sq)
                    if pending is not None:
                        issue_mel(*pending)
                    pending = (sq_tiles, b, tt)
            issue_mel(*pending)

            for g in range(8):
                q0, q1 = _QPAIRS[g]
                rp = resp.tile([128, BPC * N_MELS], F32, tag="res", name="respsum")
                nc.tensor.matmul(rp, lhsT=rb_t[g][0],
                                 rhs=logmel[:, q0, :],
                                 start=True, stop=False)
                nc.tensor.matmul(rp, lhsT=rb_t[g][1],
                                 rhs=logmel[:, q1, :],
                                 start=False, stop=True)
                ot = otpool.tile([128, BPC * N_MELS], F32, tag="ot", name="ot")
                nc.vector.tensor_copy(out=ot, in_=rp)
                for b in range(BPC):
                    nc.sync.dma_start(
                        out=out[b, g * 128:(g + 1) * 128, :],
                        in_=ot[:, b * N_MELS:(b + 1) * N_MELS],
                    )
    return nc


def _host_prep(waveform, stft_weights, mel_filters):
    wv = np.ascontiguousarray(waveform, dtype=np.float32)
    xp = np.pad(wv, ((0, 0), (PAD, PAD)), mode="reflect")  # [B, 481024]
    need = HOP * (TFR - 1) + 512 + 512  # max index reached by XtB + 1
    xz = np.zeros((B, need), np.float32)
    xz[:, : xp.shape[1]] = xp
    sb = xz.strides[0]
    xta = np.lib.stride_tricks.as_strided(
        xz, shape=(B, 512, TFR), strides=(sb, 4, HOP * 4))
    xtb = np.lib.stride_tricks.as_strided(
        xz[:, 512:], shape=(B, 512, TFR), strides=(sb, 4, HOP * 4))
    xta = np.ascontiguousarray(xta).reshape(B, 4, 128, TFR)
    xtb = np.ascontiguousarray(xtb).reshape(B, 4, 128, TFR)

    w = np.ascontiguousarray(stft_weights, dtype=np.float32)  # [1026, 1024]
    rows = list(range(0, NBINS)) + list(range(NBINS + 1, NBINS + 512))
    assert len(rows) == 1024
    wp = w[rows]                                   # [1024 packed bins, 1024 n]
    wtile = np.ascontiguousarray(wp.T).reshape(8, 128, 1024)

    mf = np.ascontiguousarray(mel_filters, dtype=np.float32)  # [513, 64]
    f_of_i = np.array([i if i < NBINS else i - 512 for i in range(1024)])
    mexp = mf[f_of_i].astype(ml_dtypes.bfloat16).reshape(8, 128, N_MELS)
    return xta, xtb, wtile, mexp


def kernel(waveform, stft_weights, mel_filters):
    global LAST_RESULTS
    xta, xtb, wtile, mexp = _host_prep(waveform, stft_weights, mel_filters)
    nc = _build_bass()
    in_maps = []
    for i in range(NCORES):
        in_maps.append({
            "xa": np.ascontiguousarray(xta[i * BPC:(i + 1) * BPC]),
            "xb": np.ascontiguousarray(xtb[i * BPC:(i + 1) * BPC]),
            "wt": wtile,
            "mexp": mexp,
            "rblk": _RBLOCKS,
        })
    nc.compile()
    res = run_bass_kernel_spmd(nc, in_maps, list(range(NCORES)), trace=TRACE)
    LAST_RESULTS = res
    out = np.concatenate([r["out"] for r in res.results], axis=0)
    return out.reshape(B, 1, SPECW, N_MELS).astype(np.float32)



# revision 8
# speedup vs baseline: 1.5822x; 1.5822x over previous
"""Trainium2 Bass kernel for nn_HFMelSpectrogram (fp8 DoubleRow version).

Pipeline (per core, 4 batches of the 32-batch waveform):
  1. STFT-as-GEMM in fp8e4m3 with perf_mode=DoubleRow: the PE array is
     virtualized to 128x256 (2 fp8 weights per cell), so each matmul
     contracts 256 of the 1024 DFT samples.  Host packs the 1024
     nontrivial DFT rows (513 cos + 511 sin) as pair-interleaved weight
     tiles [128p, 2s, 128j] and the frame matrix as [128p, 4ck, 2s, 1024t]
     with sample index n = 256*ck + 128*s + p.  Scales (x*8, W*32) keep
     fp8e4m3 (max 240) well fed.
  2. Square (PSUM -> SBUF fp8, scaled by 2^-10 before squaring), split
     between ScalarE and VectorE so neither becomes the critical path.
  3. Mel projection, also fp8 DoubleRow: stationary mexp pairs
     [128, 2s, 64m] (i-block pairs), moving sq pairs -> psum [64m, 512t].
     The cos^2+sin^2 power sum is absorbed into the GEMM contraction.
  4. Ln on ScalarE (scale 1/64 undoes the fp8 scaling; 10/log10 is folded
     into the resize matrix) -> logmel fp16 [64m, 1024t].
  5. Transpose logmel to [128t, ...] via the DMA XBAR (dma_start_transpose,
     zero engine cost), then bilinear height-resize 1000 -> 1024 as a
     banded fp16 GEMM with the 4 batches packed in the free dim.
"""

import numpy as np
import ml_dtypes

import concourse.bass as bass
import concourse.bacc as bacc
import concourse.tile as tile
import concourse.mybir as mybir
from concourse.bass_utils import run_bass_kernel_spmd

F32 = mybir.dt.float32
F16 = mybir.dt.float16
F8 = mybir.dt.float8e4
E4 = ml_dtypes.float8_e4m3
DR = mybir.MatmulPerfMode.DoubleRow

N_FFT = 1024
HOP = 480
NB_MAX = 1000      # frames kept by the reference
N_MELS = 64
SPECW = 1024       # output height after resize
NBINS = 513
B, L = 32, 480000
NCORES = 8
BPC = B // NCORES  # batches per core
TFR = 1024         # padded frame count (frames >= 1000 are zeroed via R)
PAD = N_FFT // 2

SX = 8.0           # waveform fp8 scale
SW = 32.0          # DFT weight fp8 scale
SQS = 2.0 ** -11   # pre-square scale: sq = (spec*SQS)^2 = spec_true^2/64
SM = 1024.0        # mel filter fp8 scale
# mel_psum = (SX*SW*SQS)^2 * SM * mel_true = 16 * mel_true
LN_SCALE = 1.0 / 16.0

# Set by test harness to collect a profile; harness default leaves it off.
TRACE = False
LAST_RESULTS = None


def _resize_blocks():
    """Banded resize matrix blocks, f32 coords matching the reference."""
    scale = np.float32((NB_MAX - 1) / (SPECW - 1))
    pos = np.arange(SPECW, dtype=np.float32) * scale
    h0 = np.clip(np.floor(pos).astype(np.int64), 0, NB_MAX - 1)
    frac = (pos - h0.astype(np.float32)).astype(np.float64)
    h1 = np.minimum(h0 + 1, NB_MAX - 1)
    c = 10.0 / np.log(10.0)
    r = np.zeros((TFR, SPECW), np.float64)
    cols = np.arange(SPECW)
    r[h0, cols] += (1.0 - frac) * c
    r[h1, cols] += frac * c
    blocks = np.zeros((8, 2, 128, 128), np.float16)
    qpairs = []
    for g in range(8):
        sub = r[:, g * 128:(g + 1) * 128]
        rows = np.nonzero(sub.any(axis=1))[0]
        qs = sorted({int(q) for q in rows // 128})
        assert 1 <= len(qs) <= 2, qs
        q0 = qs[0]
        q1 = qs[1] if len(qs) > 1 else min(q0 + 1, 7)
        blocks[g, 0] = sub[q0 * 128:(q0 + 1) * 128].astype(np.float16)
        blocks[g, 1] = sub[q1 * 128:(q1 + 1) * 128].astype(np.float16)
        qpairs.append((q0, q1))
    return blocks, qpairs


_RBLOCKS, _QPAIRS = _resize_blocks()


def _build_bass():
    nc = bacc.Bacc("TRN2", target_bir_lowering=False, debug=False,
                   num_devices=NCORES)
    xt = nc.declare_dram_parameter("xt", [BPC, 128, 4, 2, TFR], F8,
                                   isOutput=False)
    wt = nc.declare_dram_parameter("wt", [8, 128, 4, 2, 128], F8,
                                   isOutput=False)
    mexp = nc.declare_dram_parameter("mexp", [4, 128, 2, N_MELS], F8,
                                     isOutput=False)
    rblk = nc.declare_dram_parameter("rblk", [8, 2, 128, 128], F16,
                                     isOutput=False)
    out = nc.declare_dram_parameter("out", [BPC, SPECW, N_MELS], F32,
                                    isOutput=True)

    with tile.TileContext(nc) as tc:
        with (
            tc.tile_pool(name="consts", bufs=1) as consts,
            tc.tile_pool(name="xt", bufs=1) as xpool,
            tc.tile_pool(name="sq", bufs=2) as sqpool,
            tc.tile_pool(name="lm", bufs=1) as lmpool,
            tc.tile_pool(name="lmT", bufs=1) as lmTpool,
            tc.tile_pool(name="ot", bufs=2) as otpool,
            tc.tile_pool(name="specp", bufs=4, space="PSUM") as specp,
            tc.tile_pool(name="melp", bufs=2, space="PSUM") as melp,
            tc.tile_pool(name="resp", bufs=2, space="PSUM") as resp,
        ):
            # warmup fodder (never read back): keeps the PE HAM window busy
            # while the first input DMAs land, so real matmuls start warm.
            wu = consts.tile([128, 2, 512], F8, tag="wu", name="wu")
            nc.vector.memset(wu, 0.0)

            wt_t = []
            for m in range(8):
                t = consts.tile([128, 4, 2, 128], F8, tag=f"wt{m}",
                                name=f"wt{m}")
                nc.gpsimd.dma_start(out=t, in_=wt[m])
                wt_t.append(t)
            mexp_t = []
            for q in range(4):
                t = consts.tile([128, 2, N_MELS], F8, tag=f"me{q}",
                                name=f"me{q}")
                nc.gpsimd.dma_start(out=t, in_=mexp[q])
                mexp_t.append(t)
            rb_t = []
            for g in range(8):
                pair = []
                for j in range(2):
                    t = consts.tile([128, 128], F16, tag=f"rb{g}_{j}",
                                    name=f"rb{g}_{j}")
                    nc.gpsimd.dma_start(out=t, in_=rblk[g, j])
                    pair.append(t)
                rb_t.append(pair)
            eps_t = consts.tile([128, 1], F32, tag="eps", name="eps")
            nc.vector.memset(eps_t, 1e-10)

            xt_t = []
            for b in range(BPC):
                t = xpool.tile([128, 4, 2, TFR], F8, tag=f"xt{b}",
                               name=f"xt{b}")
                nc.sync.dma_start(out=t, in_=xt[b])
                xt_t.append(t)

            sq_t = [[None] * 4 for _ in range(BPC)]
            lm_t = []
            for b in range(BPC):
                lm_t.append(lmpool.tile([N_MELS, TFR], F16, tag=f"lm{b}",
                                        name=f"lm{b}"))
            # transposed logmel: [t_local, t_chunk q, batch, mel]
            lmT = lmTpool.tile([128, 8, BPC, N_MELS], F16, tag="lmT",
                               name="lmT")

            # PE warmup: ~4us of dummy matmuls racing the first input DMA.
            wups = specp.tile([128, 512], F32, tag="spec", name="wups")
            for i in range(18):
                nc.tensor.matmul(wups, lhsT=wu[:, :, :128], rhs=wu,
                                 start=True, stop=True, perf_mode=DR,
                                 skip_group_check=True)

            def issue_mel(b):
                for tc_ in range(2):
                    mp = melp.tile([N_MELS, 512], F32, tag="mel",
                                   name="melpsum")
                    for q in range(4):
                        nc.tensor.matmul(
                            mp,
                            lhsT=mexp_t[q],
                            rhs=sq_t[b][q][:, :, tc_ * 512:(tc_ + 1) * 512],
                            start=(q == 0),
                            stop=(q == 3),
                            perf_mode=DR,
                        )
                    nc.scalar.activation(
                        out=lm_t[b][:, tc_ * 512:(tc_ + 1) * 512],
                        in_=mp,
                        func=mybir.ActivationFunctionType.Ln,
                        bias=eps_t[:N_MELS, :],
                        scale=LN_SCALE,
                    )
                for tg in range(8):
                    nc.sync.dma_start_transpose(
                        out=lmT[:, tg, b, :],
                        in_=lm_t[b][:, tg * 128:(tg + 1) * 128],
                    )

            pending = None
            for b in range(BPC):
                for m in range(8):
                    ps0 = specp.tile([128, 512], F32, tag="spec",
                                     name="specpsum")
                    ps1 = specp.tile([128, 512], F32, tag="spec",
                                     name="specpsum")
                    pst = (ps0, ps1)
                    for ck in range(4):
                        for tc_ in range(2):
                            nc.tensor.matmul(
                                pst[tc_],
                                lhsT=wt_t[m][:, ck],
                                rhs=xt_t[b][:, ck, :,
                                            tc_ * 512:(tc_ + 1) * 512],
                                start=(ck == 0),
                                stop=(ck == 3),
                                perf_mode=DR,
                                skip_group_check=True,
                            )
                    q, s = divmod(m, 2)
                    if s == 0:
                        sq_t[b][q] = sqpool.tile([128, 2, TFR], F8,
                                                 tag=f"sq{q}", name=f"sq{q}")
                    # squares on ScalarE (walrus rejects DVE dual-PSUM reads)
                    for tc_ in range(2):
                        nc.scalar.activation(
                            out=sq_t[b][q][:, s, tc_ * 512:(tc_ + 1) * 512],
                            in_=pst[tc_],
                            func=mybir.ActivationFunctionType.Square,
                            scale=SQS,
                        )
                    if m == 1 and pending is not None:
                        issue_mel(pending)
                pending = b
            issue_mel(pending)

            for g in range(8):
                q0, q1 = _QPAIRS[g]
                rp = resp.tile([128, BPC * N_MELS], F32, tag="res",
                               name="respsum")
                nc.tensor.matmul(rp, lhsT=rb_t[g][0], rhs=lmT[:, q0, :, :],
                                 start=True, stop=False)
                nc.tensor.matmul(rp, lhsT=rb_t[g][1], rhs=lmT[:, q1, :, :],
                                 start=False, stop=True)
                ot = otpool.tile([128, BPC * N_MELS], F32, tag="ot",
                                 name="ot")
                nc.vector.tensor_copy(out=ot, in_=rp)
                nc.gpsimd.dma_start(
                    out=out[:, g * 128:(g + 1) * 128, :].rearrange(
                        "b h m -> h b m"),
                    in_=ot,
                )
    return nc


def _host_prep(waveform, stft_weights, mel_filters):
    wv = np.ascontiguousarray(waveform, dtype=np.float32)
    xp = np.pad(wv, ((0, 0), (PAD, PAD)), mode="reflect")  # [B, 481024]
    need = HOP * (TFR - 1) + N_FFT  # 492064: max sample index + 1
    xz = np.zeros((B, need), np.float32)
    xz[:, : xp.shape[1]] = xp
    xq = (xz * SX).astype(E4)  # quantize once, then view strided
    sb = xq.strides[0]
    xt8 = np.lib.stride_tricks.as_strided(
        xq, shape=(B, 128, 4, 2, TFR), strides=(sb, 1, 256, 128, HOP))
    xt8 = np.ascontiguousarray(xt8)

    w = np.ascontiguousarray(stft_weights, dtype=np.float32)  # [1026, 1024]
    rows = list(range(0, NBINS)) + list(range(NBINS + 1, NBINS + 512))
    assert len(rows) == 1024
    wp = (w[rows] * SW).astype(E4)          # [1024 i, 1024 n]
    # i = 128*m + j ; n = 256*ck + 128*s + p  ->  [m, p, ck, s, j]
    wt8 = np.ascontiguousarray(
        wp.reshape(8, 128, 4, 2, 128).transpose(0, 4, 2, 3, 1))

    mf = np.ascontiguousarray(mel_filters, dtype=np.float32)  # [513, 64]
    f_of_i = np.array([i if i < NBINS else i - 512 for i in range(1024)])
    mexp = (mf[f_of_i] * SM).astype(E4)      # [1024 i, 64]
    # i = 128*(2q+s) + p -> [q, p, s, mm]
    mexp8 = np.ascontiguousarray(
        mexp.reshape(4, 2, 128, N_MELS).transpose(0, 2, 1, 3))
    return xt8, wt8, mexp8


def kernel(waveform, stft_weights, mel_filters):
    global LAST_RESULTS
    xt8, wt8, mexp8 = _host_prep(waveform, stft_weights, mel_filters)
    nc = _build_bass()
    in_maps = []
    for i in range(NCORES):
        in_maps.append({
            "xt": np.ascontiguousarray(xt8[i * BPC:(i + 1) * BPC]),
            "wt": wt8,
            "mexp": mexp8,
            "rblk": _RBLOCKS,
        })
    nc.compile()
    res = run_bass_kernel_spmd(nc, in_maps, list(range(NCORES)), trace=TRACE)
    LAST_RESULTS = res
    out = np.concatenate([r["out"] for r in res.results], axis=0)
    return out.reshape(B, 1, SPECW, N_MELS).astype(np.float32)


# revision 9
# speedup vs baseline: 1.8074x; 1.1423x over previous
"""Trainium2 Bass kernel for nn_HFMelSpectrogram (fp8 DoubleRow version).

Pipeline (per core, 4 batches of the 32-batch waveform):
  1. STFT-as-GEMM in fp8e4m3 with perf_mode=DoubleRow: the PE array is
     virtualized to 128x256 (2 fp8 weights per cell), so each matmul
     contracts 256 of the 1024 DFT samples at 2 MACs/cell/cycle.  Host
     packs the 1024 nontrivial DFT rows (513 cos + 511 sin) as
     pair-interleaved weight tiles [128p, 2s, 128j] and the frame matrix
     as [128p, 4ck, 2s, 1024t] with sample index n = 256*ck + 128*s + p.
     Scales (x*8, W*32) keep fp8e4m3 (max 240) well fed.
  2. Square on ScalarE (PSUM -> SBUF fp8, one [128,1024] op per m-tile;
     the 2^-11 input scale keeps spec^2/64 <= ~222 under the fp8 max).
  3. Mel projection, also fp8 DoubleRow: stationary mexp pairs
     [128, 2s, 64m] (i-block pairs), moving sq pairs -> psum [64m, 512t].
     The cos^2+sin^2 power sum is absorbed into the GEMM contraction.
     Issued in two chunks (q0-1 after m=5, q2-3 after the batch) so the
     accumulation overlaps the next batch's STFT.
  4. Ln on ScalarE (scale 1/16 undoes the fp8 scaling; 10/log10 is folded
     into the resize matrix) -> logmel fp16 [64m, 1024t].
  5. Transpose logmel to [128t, ...] via the DMA XBAR (dma_start_transpose,
     one [64, 512] op per half-batch, zero engine cost), then bilinear
     height-resize 1000 -> 1024 as a banded fp16 GEMM with the 4 batches
     packed in the free dim.
"""

import numpy as np
import ml_dtypes

import concourse.bass as bass
import concourse.bacc as bacc
import concourse.tile as tile
import concourse.mybir as mybir
from concourse.bass_utils import run_bass_kernel_spmd

F32 = mybir.dt.float32
F16 = mybir.dt.float16
F8 = mybir.dt.float8e4
E4 = ml_dtypes.float8_e4m3
DR = mybir.MatmulPerfMode.DoubleRow

N_FFT = 1024
HOP = 480
NB_MAX = 1000      # frames kept by the reference
N_MELS = 64
SPECW = 1024       # output height after resize
NBINS = 513
B, L = 32, 480000
NCORES = 8
BPC = B // NCORES  # batches per core
TFR = 1024         # padded frame count (frames >= 1000 are zeroed via R)
PAD = N_FFT // 2

SX = 8.0           # waveform fp8 scale
SW = 32.0          # DFT weight fp8 scale
SQS = 2.0 ** -11   # pre-square scale: sq = (spec*SQS)^2 = spec_true^2/64
SM = 1024.0        # mel filter fp8 scale
# mel_psum = (SX*SW*SQS)^2 * SM * mel_true = 16 * mel_true
LN_SCALE = 1.0 / 16.0

# Set by test harness to collect a profile; harness default leaves it off.
TRACE = False
LAST_RESULTS = None


def _resize_blocks():
    """Banded resize matrix blocks, f32 coords matching the reference."""
    scale = np.float32((NB_MAX - 1) / (SPECW - 1))
    pos = np.arange(SPECW, dtype=np.float32) * scale
    h0 = np.clip(np.floor(pos).astype(np.int64), 0, NB_MAX - 1)
    frac = (pos - h0.astype(np.float32)).astype(np.float64)
    h1 = np.minimum(h0 + 1, NB_MAX - 1)
    c = 10.0 / np.log(10.0)
    r = np.zeros((TFR, SPECW), np.float64)
    cols = np.arange(SPECW)
    r[h0, cols] += (1.0 - frac) * c
    r[h1, cols] += frac * c
    blocks = np.zeros((8, 2, 128, 128), np.float16)
    qpairs = []
    for g in range(8):
        sub = r[:, g * 128:(g + 1) * 128]
        rows = np.nonzero(sub.any(axis=1))[0]
        qs = sorted({int(q) for q in rows // 128})
        assert 1 <= len(qs) <= 2, qs
        q0 = qs[0]
        q1 = qs[1] if len(qs) > 1 else min(q0 + 1, 7)
        blocks[g, 0] = sub[q0 * 128:(q0 + 1) * 128].astype(np.float16)
        blocks[g, 1] = sub[q1 * 128:(q1 + 1) * 128].astype(np.float16)
        qpairs.append((q0, q1))
    return blocks, qpairs


_RBLOCKS, _QPAIRS = _resize_blocks()


def _build_bass():
    nc = bacc.Bacc("TRN2", target_bir_lowering=False, debug=False,
                   num_devices=NCORES)
    xt = nc.declare_dram_parameter("xt", [BPC, 128, 4, 2, TFR], F8,
                                   isOutput=False)
    wt = nc.declare_dram_parameter("wt", [8, 128, 4, 2, 128], F8,
                                   isOutput=False)
    mexp = nc.declare_dram_parameter("mexp", [4, 128, 2, N_MELS], F8,
                                     isOutput=False)
    rblk = nc.declare_dram_parameter("rblk", [8, 2, 128, 128], F16,
                                     isOutput=False)
    out = nc.declare_dram_parameter("out", [BPC, SPECW, N_MELS], F32,
                                    isOutput=True)

    with tile.TileContext(nc) as tc:
        with (
            tc.tile_pool(name="consts", bufs=1) as consts,
            tc.tile_pool(name="xt", bufs=1) as xpool,
            tc.tile_pool(name="sq", bufs=2) as sqpool,
            tc.tile_pool(name="lm", bufs=1) as lmpool,
            tc.tile_pool(name="lmT", bufs=1) as lmTpool,
            tc.tile_pool(name="ot", bufs=2) as otpool,
            tc.tile_pool(name="specp", bufs=3, space="PSUM") as specp,
            tc.tile_pool(name="melp", bufs=2, space="PSUM") as melp,
        ):
            # warmup fodder (never read back): keeps the PE HAM window busy
            # while the first input DMAs land, so real matmuls start warm.
            wu = consts.tile([128, 2, 512], F8, tag="wu", name="wu")
            nc.vector.memset(wu, 0.0)
            eps_t = consts.tile([128, 1], F32, tag="eps", name="eps")
            nc.vector.memset(eps_t, 1e-10)
            # activation-table preload (Square then Ln) during the head DMAs
            dmy = consts.tile([1, 1], F32, tag="dmy", name="dmy")
            nc.scalar.activation(out=dmy, in_=eps_t[:1, :],
                                 func=mybir.ActivationFunctionType.Square)
            nc.scalar.activation(out=dmy, in_=eps_t[:1, :],
                                 func=mybir.ActivationFunctionType.Ln,
                                 bias=eps_t[:1, :])

            wt_t = []
            for m in range(8):
                t = consts.tile([128, 4, 2, 128], F8, tag=f"wt{m}",
                                name=f"wt{m}")
                nc.gpsimd.dma_start(out=t, in_=wt[m])
                wt_t.append(t)
            mexp_t = []
            for q in range(4):
                t = consts.tile([128, 2, N_MELS], F8, tag=f"me{q}",
                                name=f"me{q}")
                nc.gpsimd.dma_start(out=t, in_=mexp[q])
                mexp_t.append(t)
            rb_t = []
            for g in range(8):
                pair = []
                for j in range(2):
                    t = consts.tile([128, 128], F16, tag=f"rb{g}_{j}",
                                    name=f"rb{g}_{j}")
                    nc.gpsimd.dma_start(out=t, in_=rblk[g, j])
                    pair.append(t)
                rb_t.append(pair)

            xt_t = []
            for b in range(BPC):
                t = xpool.tile([128, 4, 2, TFR], F8, tag=f"xt{b}",
                               name=f"xt{b}")
                nc.sync.dma_start(out=t, in_=xt[b])
                xt_t.append(t)

            sq_t = [[None] * 4 for _ in range(BPC)]
            lm_t = []
            for b in range(BPC):
                lm_t.append(lmpool.tile([N_MELS, TFR], F16, tag=f"lm{b}",
                                        name=f"lm{b}"))
            # transposed logmel: [t_local, t_chunk q, batch, mel]
            lmT = lmTpool.tile([128, 8, BPC, N_MELS], F16, tag="lmT",
                               name="lmT")

            # PE warmup: ~4us of dummy matmuls racing the first input DMA.
            wups = specp.tile([128, TFR], F32, tag="spec", name="wups")
            for i in range(18):
                nc.tensor.matmul(wups[:, :512], lhsT=wu[:, :, :128], rhs=wu,
                                 start=True, stop=True, perf_mode=DR,
                                 skip_group_check=True)

            mel_ps = [None] * BPC

            def issue_mel_chunk1(b):
                """q0,q1 contributions for both t-halves."""
                mel_ps[b] = [melp.tile([N_MELS, 512], F32, tag="mel",
                                       name=f"mel{b}_{tc_}")
                             for tc_ in range(2)]
                for tc_ in range(2):
                    for q in range(2):
                        nc.tensor.matmul(
                            mel_ps[b][tc_],
                            lhsT=mexp_t[q],
                            rhs=sq_t[b][q][:, :, tc_ * 512:(tc_ + 1) * 512],
                            start=(q == 0),
                            stop=False,
                            perf_mode=DR,
                            skip_group_check=True,
                        )

            def issue_mel_chunk2(b):
                """q2,q3 + Ln + transpose, per t-half."""
                for tc_ in range(2):
                    for q in range(2, 4):
                        nc.tensor.matmul(
                            mel_ps[b][tc_],
                            lhsT=mexp_t[q],
                            rhs=sq_t[b][q][:, :, tc_ * 512:(tc_ + 1) * 512],
                            start=False,
                            stop=(q == 3),
                            perf_mode=DR,
                            skip_group_check=True,
                        )
                    nc.scalar.activation(
                        out=lm_t[b][:, tc_ * 512:(tc_ + 1) * 512],
                        in_=mel_ps[b][tc_],
                        func=mybir.ActivationFunctionType.Ln,
                        bias=eps_t[:N_MELS, :],
                        scale=LN_SCALE,
                    )
                    nc.sync.dma_start_transpose(
                        out=lmT[:, tc_ * 4:(tc_ + 1) * 4, b, :],
                        in_=lm_t[b][:, tc_ * 512:(tc_ + 1) * 512],
                    )

            pending = None
            for b in range(BPC):
                for m in range(8):
                    ps = specp.tile([128, TFR], F32, tag="spec",
                                    name="specpsum")
                    for ck in range(4):
                        for tc_ in range(2):
                            nc.tensor.matmul(
                                ps[:, tc_ * 512:(tc_ + 1) * 512],
                                lhsT=wt_t[m][:, ck],
                                rhs=xt_t[b][:, ck, :,
                                            tc_ * 512:(tc_ + 1) * 512],
                                start=(ck == 0),
                                stop=(ck == 3),
                                perf_mode=DR,
                                skip_group_check=True,
                            )
                    q, s = divmod(m, 2)
                    if s == 0:
                        sq_t[b][q] = sqpool.tile([128, 2, TFR], F8,
                                                 tag=f"sq{q}", name=f"sq{q}")
                    nc.scalar.activation(
                        out=sq_t[b][q][:, s, :],
                        in_=ps,
                        func=mybir.ActivationFunctionType.Square,
                        scale=SQS,
                    )
                    if m == 1 and pending is not None:
                        issue_mel_chunk2(pending)
                    if m == 6:
                        issue_mel_chunk1(b)
                pending = b
            issue_mel_chunk2(pending)

            for g in range(8):
                q0, q1 = _QPAIRS[g]
                rp = specp.tile([128, TFR], F32, tag="spec", name="respsum")
                nc.tensor.matmul(rp[:, :BPC * N_MELS], lhsT=rb_t[g][0],
                                 rhs=lmT[:, q0, :, :],
                                 start=True, stop=False)
                nc.tensor.matmul(rp[:, :BPC * N_MELS], lhsT=rb_t[g][1],
                                 rhs=lmT[:, q1, :, :],
                                 start=False, stop=True)
                ot = otpool.tile([128, BPC * N_MELS], F32, tag="ot",
                                 name="ot")
                nc.vector.tensor_copy(out=ot, in_=rp[:, :BPC * N_MELS])
                nc.gpsimd.dma_start(
                    out=out[:, g * 128:(g + 1) * 128, :].rearrange(
                        "b h m -> h b m"),
                    in_=ot,
                )
    return nc


def _host_prep(waveform, stft_weights, mel_filters):
    wv = np.ascontiguousarray(waveform, dtype=np.float32)
    xp = np.pad(wv, ((0, 0), (PAD, PAD)), mode="reflect")  # [B, 481024]
    need = HOP * (TFR - 1) + N_FFT  # 492064: max sample index + 1
    xz = np.zeros((B, need), np.float32)
    xz[:, : xp.shape[1]] = xp
    xq = (xz * SX).astype(E4)  # quantize once, then view strided
    sb = xq.strides[0]
    xt8 = np.lib.stride_tricks.as_strided(
        xq, shape=(B, 128, 4, 2, TFR), strides=(sb, 1, 256, 128, HOP))
    xt8 = np.ascontiguousarray(xt8)

    w = np.ascontiguousarray(stft_weights, dtype=np.float32)  # [1026, 1024]
    rows = list(range(0, NBINS)) + list(range(NBINS + 1, NBINS + 512))
    assert len(rows) == 1024
    wp = (w[rows] * SW).astype(E4)          # [1024 i, 1024 n]
    # i = 128*m + j ; n = 256*ck + 128*s + p  ->  [m, p, ck, s, j]
    wt8 = np.ascontiguousarray(
        wp.reshape(8, 128, 4, 2, 128).transpose(0, 4, 2, 3, 1))

    mf = np.ascontiguousarray(mel_filters, dtype=np.float32)  # [513, 64]
    f_of_i = np.array([i if i < NBINS else i - 512 for i in range(1024)])
    mexp = (mf[f_of_i] * SM).astype(E4)      # [1024 i, 64]
    # i = 128*(2q+s) + p -> [q, p, s, mm]
    mexp8 = np.ascontiguousarray(
        mexp.reshape(4, 2, 128, N_MELS).transpose(0, 2, 1, 3))
    return xt8, wt8, mexp8


def kernel(waveform, stft_weights, mel_filters):
    global LAST_RESULTS
    xt8, wt8, mexp8 = _host_prep(waveform, stft_weights, mel_filters)
    nc = _build_bass()
    in_maps = []
    for i in range(NCORES):
        in_maps.append({
            "xt": np.ascontiguousarray(xt8[i * BPC:(i + 1) * BPC]),
            "wt": wt8,
            "mexp": mexp8,
            "rblk": _RBLOCKS,
        })
    nc.compile()
    res = run_bass_kernel_spmd(nc, in_maps, list(range(NCORES)), trace=TRACE)
    LAST_RESULTS = res
    out = np.concatenate([r["out"] for r in res.results], axis=0)
    return out.reshape(B, 1, SPECW, N_MELS).astype(np.float32)
